# revision 1
# baseline (speedup 1.0000x reference)
"""Chamfer loss kernel for Trainium2, batch-parallel over 8 NeuronCores.

Per core (one batch element b):
  gts = src_points[b] @ R^T + t          (on device, bf16-pair matmul)
  P[i,j] = |gts_i|^2 + |recon_j|^2 - 2 gts_i . recon_j
  loss_b = sum_j min_i P + sum_i min_j P
Host sums the 8 partial losses.

Structure:
- The -2*g.p cross terms and the yy_j norm enter each distance tile via a
  single augmented K=10 bf16 matmul (cross terms use a bf16 hi/lo pair
  decomposition, hi*hi + hi*lo + lo*hi, ~2^-18 relative accuracy; the
  norms are single bf16 since their error is row/column-structured and
  cancels to ~1e-4 in the summed loss) at 1 PE cycle/row.
- The xx_i norm is added during the PSUM->SBUF staging copy as the ACT
  engine's per-partition bias, so the xx computation never gates the
  main matmul pipeline.
- The staged bf16 tiles are reduced by DVE in 2x mode: a running
  elementwise min for the per-column mins, and a batched binary min-tree
  (two row blocks at a time) for the per-row mins.
- Per-column mins are finished with PE transposes + free-axis folds, and
  everything is summed with a final ones-matmul across partitions.
"""

import os

# the axon client here has no NTFF profile hook; a stray BASS_TRACE=1 in the
# environment would crash run_bass_kernel_spmd on a missing import
os.environ["BASS_NEVER_TRACE"] = "1"

import ml_dtypes
import numpy as np

import concourse.bacc as bacc
import concourse.bass as bass
import concourse.mybir as mybir
import concourse.tile as tile
from concourse.bass_utils import run_bass_kernel_spmd

F32 = mybir.dt.float32
BF16 = mybir.dt.bfloat16
ALU = mybir.AluOpType
AX = mybir.AxisListType
AF = mybir.ActivationFunctionType

N_CORES = 8
NPTS = 4096          # points per set (both gts and recon)
NBLK = NPTS // 128   # 32 row blocks
HALF = 2048          # P tile free width (4 PSUM banks)

_CACHE = {}
LAST_RESULTS = None


def _build_kernel():
    nc = bacc.Bacc("TRN2", target_bir_lowering=False, debug=False)

    srcT = nc.declare_dram_parameter("srcT", [4, NPTS], F32, isOutput=False)
    reconT = nc.declare_dram_parameter("reconT", [4, NPTS], F32, isOutput=False)
    taug = nc.declare_dram_parameter("taug", [4, 4], F32, isOutput=False)
    ident = nc.declare_dram_parameter("ident", [128, 128], BF16, isOutput=False)
    cnorm = nc.declare_dram_parameter("cnorm", [8, 2], BF16, isOutput=False)
    cscal = nc.declare_dram_parameter("cscal", [8, 1], F32, isOutput=False)
    cones = nc.declare_dram_parameter("cones", [128, 1], F32, isOutput=False)
    loss = nc.declare_dram_parameter("loss", [1, 1], F32, isOutput=True)

    with tile.TileContext(nc) as tc:
        with tc.tile_pool(name="sb", bufs=1) as sb:
            prep_pool = tc.alloc_tile_pool(name="prep", bufs=1)
            # ---- phase 0: load inputs (chunked over DMA queues) ---------
            pts = prep_pool.tile([8, NPTS], F32) # rows 0-3 gts_aug, 4-7 recon_aug
            for c in range(4):
                cs = slice(c * 1024, (c + 1) * 1024)
                nc.sync.dma_start(out=pts[0:4, cs], in_=srcT[:, cs])
                nc.sync.dma_start(out=pts[4:8, cs], in_=reconT[:, cs])

            taug_sb = sb.tile([4, 4], F32)
            nc.sync.dma_start(out=taug_sb[:, :], in_=taug[:, :])
            ident_sb = sb.tile([128, 128], BF16)
            nc.sync.dma_start(out=ident_sb[:, :], in_=ident[:, :])
            norm_ones = sb.tile([8, 2], BF16)
            nc.sync.dma_start(out=norm_ones[:, :], in_=cnorm[:, :])
            scal = sb.tile([8, 1], F32)
            nc.sync.dma_start(out=scal[:, :], in_=cscal[:, :])
            ones128 = sb.tile([128, 1], F32)
            nc.sync.dma_start(out=ones128[:, :], in_=cones[:, :])

            # PE warm-up: ~40 tiny matmuls on the identity while inputs
            # load, so the transform/norm matmuls run at full PE clock
            with tc.tile_pool(name="warm_ps", bufs=1, space="PSUM") as wpp:
                warm_ps = wpp.tile([128, 128], F32)
                for _ in range(40):
                    nc.tensor.matmul(warm_ps[:, :], lhsT=ident_sb[:, :],
                                     rhs=ident_sb[:, :], start=True,
                                     stop=True)

            # ---- phase 1: operand prep ----------------------------------
            # bf16 hi/lo of the transform and of the source points
            th = sb.tile([4, 4], BF16)
            tl = sb.tile([4, 4], BF16)
            nc.vector.tensor_copy(th[:, :], taug_sb[:, :])
            nc.vector.scalar_tensor_tensor(tl[:, :], taug_sb[:, :], 1.0,
                                           th[:, :], ALU.mult, ALU.subtract)
            s_hi = prep_pool.tile([4, NPTS], BF16)
            s_lo = prep_pool.tile([4, NPTS], BF16)
            nc.vector.tensor_copy(s_hi[:, :], pts[0:4, :])
            nc.vector.scalar_tensor_tensor(s_lo[:, :], pts[0:4, :], 1.0,
                                           s_hi[:, :], ALU.mult, ALU.subtract)
            tlhs = sb.tile([12, 4], BF16)
            nc.sync.dma_start(out=tlhs[0:4, :], in_=th[:, :])
            nc.sync.dma_start(out=tlhs[4:8, :], in_=th[:, :])
            nc.sync.dma_start(out=tlhs[8:12, :], in_=tl[:, :])
            trhs = prep_pool.tile([12, NPTS], BF16)
            nc.sync.dma_start(out=trhs[0:4, :], in_=s_hi[:, :])
            nc.sync.dma_start(out=trhs[4:8, :], in_=s_lo[:, :])
            nc.sync.dma_start(out=trhs[8:12, :], in_=s_hi[:, :])

            # squares in single bf16: the staged distance tiles are bf16
            # anyway, and norm errors are row/column-structured, so norm
            # accuracy at bf16 level is provably negligible for the loss
            sqb = prep_pool.tile([8, NPTS], BF16)
            nxy = prep_pool.tile([2, NPTS], BF16)
            xxT = sb.tile([128, NBLK], BF16)
            # full 8 rows (ACT needs 32-aligned partition bases); the src
            # rows squared here are dummies, overwritten from gts below
            nc.scalar.activation(sqb[:, :], pts[:, :], AF.Square)

            # transform: gts^T rows 0-2 (+ intact ones row 3)
            with tc.tile_pool(name="gts_ps", bufs=1, space="PSUM") as gpp:
                gts_ps = gpp.tile([4, NPTS], F32)
                for c in range(NPTS // 512):
                    cs = slice(c * 512, (c + 1) * 512)
                    nc.tensor.matmul(gts_ps[:, cs], lhsT=tlhs[:, :],
                                     rhs=trhs[:, cs], start=True, stop=True)
                nc.scalar.copy(pts[0:4, :], gts_ps[:, :])
                nc.scalar.activation(sqb[0:4, :], gts_ps[:, :], AF.Square)

            # bf16 hi/lo of (-2*gts | recon)
            c_hi = prep_pool.tile([8, NPTS], BF16)
            c_lo = prep_pool.tile([8, NPTS], BF16)
            nc.vector.tensor_scalar(c_hi[:, :], pts[:, :], scal[:, :], None,
                                    ALU.mult)
            nc.vector.scalar_tensor_tensor(c_lo[:, :], pts[:, :], scal[:, :],
                                           c_hi[:, :], ALU.mult, ALU.subtract)

            # xx (row 0) and yy (row 1) via one K=8 bf16 ones-matmul
            with tc.tile_pool(name="nrm_ps", bufs=1, space="PSUM") as npp:
                nrm_ps = npp.tile([2, NPTS], F32)
                for c in range(NPTS // 512):
                    cs = slice(c * 512, (c + 1) * 512)
                    nc.tensor.matmul(nrm_ps[:, cs], lhsT=norm_ones[:, :],
                                     rhs=sqb[:, cs], start=True, stop=True)
                nc.scalar.copy(nxy[:, :], nrm_ps[:, :])

            # assemble the K=10 matmul operands (SBUF->SBUF DMA row moves)
            # k 0-2: -2g_hi | p_hi   k 3-5: -2g_hi | p_lo   k 6-8: -2g_lo | p_hi
            # k 9:   1      | yy
            lhs = sb.tile([16, NPTS], BF16)
            rhs = sb.tile([16, NPTS], BF16)
            nc.sync.dma_start(out=lhs[0:3, :], in_=c_hi[0:3, :])
            nc.sync.dma_start(out=lhs[3:6, :], in_=c_hi[0:3, :])
            nc.sync.dma_start(out=lhs[6:9, :], in_=c_lo[0:3, :])
            nc.sync.dma_start(out=lhs[9:10, :], in_=c_hi[3:4, :])   # bf16 ones
            nc.sync.dma_start(out=rhs[0:3, :], in_=c_hi[4:7, :])
            nc.sync.dma_start(out=rhs[3:6, :], in_=c_lo[4:7, :])
            nc.sync.dma_start(out=rhs[6:9, :], in_=c_hi[4:7, :])
            nc.sync.dma_start(out=rhs[9:10, :], in_=nxy[1:2, :])

            # relayout xx [1, 4096] -> [128, 32] via DRAM (gates only the
            # staging copies, not the matmuls): xxT[p, b] = xx[b*128 + p]
            xx_dram = nc.dram_tensor("xx_scratch", [1, NPTS], BF16)
            nc.sync.dma_start(out=xx_dram[0:1, :], in_=nxy[0:1, :])
            nc.sync.dma_start(
                out=xxT[:, :],
                in_=xx_dram[0:1, :].rearrange("o (b p) -> (o p) b", p=128))

            prep_pool.release()

            # ---- phase 3: distance tiles + min reductions ---------------
            rmin = sb.tile([128, NBLK], F32)        # per-block row mins
            mrun = sb.tile([128, NPTS], BF16)       # running col-min over i

            with tc.tile_pool(name="stage_sb", bufs=3) as stg, \
                 tc.tile_pool(name="main_ps", bufs=2, space="PSUM") as mps:
                for ip in range(NBLK // 2):
                    # stage a PAIR of row blocks, then one batched tree
                    pb = stg.tile([128, 2 * NPTS], BF16, tag="PSB", bufs=2)
                    for q in range(2):
                        ib = 2 * ip + q
                        lw = lhs[0:10, ib * 128:(ib + 1) * 128]
                        for h in range(2):
                            pt = mps.tile([128, HALF], F32, tag="P")
                            for s in range(HALF // 512):
                                j0 = h * HALF + s * 512
                                nc.tensor.matmul(
                                    pt[:, s * 512:(s + 1) * 512], lhsT=lw,
                                    rhs=rhs[0:10, j0:j0 + 512],
                                    start=True, stop=True)
                            # stage to bf16 SBUF, adding xx_i as ACT bias
                            nc.scalar.activation(
                                pb[:, q * NPTS + h * HALF:
                                   q * NPTS + (h + 1) * HALF],
                                pt[:, :], AF.Identity,
                                bias=xxT[:, ib:ib + 1], scale=1.0)
                        # running col-min (one 4096-wide 2x op per block)
                        pslice = pb[:, q * NPTS:(q + 1) * NPTS]
                        if ib == 0:
                            nc.vector.tensor_copy(mrun[:, :], pslice)
                        else:
                            nc.vector.tensor_tensor(mrun[:, :], pslice,
                                                    mrun[:, :], ALU.min)
                    # batched row-min tree: [128, 2, w] views, both blocks
                    w = HALF
                    tr = pb.rearrange("p (b h w) -> p b h w", b=2, h=2)
                    lvl = 0
                    while w >= 128:
                        nt = stg.tile([128, 2, w], BF16, tag=f"TR{lvl}",
                                      bufs=2, name=f"tr{lvl}")
                        nc.vector.tensor_tensor(nt[:, :, :], tr[:, :, 0, :],
                                                tr[:, :, 1, :], ALU.min)
                        tr = nt.rearrange("p b (h w) -> p b h w", h=2)
                        w //= 2
                        lvl += 1
                    nc.vector.tensor_reduce(
                        rmin[:, 2 * ip:2 * ip + 2],
                        tr.rearrange("p b h w -> p b (h w)"), axis=AX.X,
                        op=ALU.min)

            # ---- phase 4: finishers -------------------------------------
            rsum = sb.tile([128, 1], F32)
            cmin = sb.tile([128, NPTS // 128], F32)
            csum = sb.tile([128, 1], F32)
            tot = sb.tile([128, 1], F32)
            loss_sb = sb.tile([1, 1], F32)

            nc.vector.tensor_reduce(rsum[:, :], rmin[:, :], axis=AX.X,
                                    op=ALU.add)

            with tc.tile_pool(name="fin_ps", bufs=4, space="PSUM") as fps:
                # 8 transposes per PSUM tile, one batched fold per group
                for g in range(NPTS // 1024):
                    tp = fps.tile([128, 1024], BF16, tag="T")
                    for c in range(8):
                        j0 = (g * 8 + c) * 128
                        nc.tensor.transpose(tp[:, c * 128:(c + 1) * 128],
                                            mrun[:, j0:j0 + 128],
                                            ident_sb[:, :])
                    nc.vector.tensor_reduce(
                        cmin[:, 8 * g:8 * g + 8],
                        tp.rearrange("p (g w) -> p g w", w=128),
                        axis=AX.X, op=ALU.min)
                nc.vector.tensor_reduce(csum[:, :], cmin[:, :], axis=AX.X,
                                        op=ALU.add)
                nc.vector.tensor_tensor(tot[:, :], rsum[:, :], csum[:, :],
                                        ALU.add)

                loss_ps = fps.tile([1, 1], F32, tag="L", bufs=1)
                nc.tensor.matmul(loss_ps[:, :], lhsT=tot[:, :],
                                 rhs=ones128[:, :], start=True, stop=True)
                nc.scalar.copy(loss_sb[:, :], loss_ps[:, :])

            nc.sync.dma_start(out=loss[:, :], in_=loss_sb[:, :])

    nc.compile()
    return nc


def _prep_core_inputs(recon_b, src_b, transform_b):
    src_aug = np.empty((4, NPTS), np.float32)
    src_aug[0:3] = src_b.T
    src_aug[3] = 1.0
    rec_aug = np.empty((4, NPTS), np.float32)
    rec_aug[0:3] = recon_b.T
    rec_aug[3] = 1.0
    R = transform_b[:3, :3]
    t = transform_b[:3, 3]
    ta = np.zeros((4, 4), np.float32)
    ta[0:3, 0:3] = R.T
    ta[3, 0:3] = t
    ta[3, 3] = 1.0
    cnorm = np.zeros((8, 2), np.float32)
    cnorm[0:3, 0] = 1.0    # xx from gts squares
    cnorm[4:7, 1] = 1.0    # yy from recon squares
    cnorm = cnorm.astype(ml_dtypes.bfloat16)
    cscal = np.zeros((8, 1), np.float32)
    cscal[0:3] = -2.0
    cscal[3] = 1.0      # aug row -> bf16 ones source
    cscal[4:7] = 1.0
    cscal[7] = 1.0
    return {
        "srcT": np.ascontiguousarray(src_aug),
        "reconT": np.ascontiguousarray(rec_aug),
        "taug": ta,
        "ident": np.eye(128).astype(ml_dtypes.bfloat16),
        "cnorm": cnorm,
        "cscal": cscal,
        "cones": np.ones((128, 1), np.float32),
    }


def kernel(recon, src_points, transform):
    global LAST_RESULTS
    recon = np.asarray(recon, np.float32)
    src_points = np.asarray(src_points, np.float32)
    transform = np.asarray(transform, np.float32)
    B = recon.shape[0]
    assert B == N_CORES

    if "nc" not in _CACHE:
        _CACHE["nc"] = _build_kernel()
    nc = _CACHE["nc"]

    in_maps = [
        _prep_core_inputs(recon[b], src_points[b], transform[b])
        for b in range(B)
    ]
    res = run_bass_kernel_spmd(nc, in_maps, list(range(N_CORES)))
    LAST_RESULTS = res
    total = np.float64(0.0)
    for r in res.results:
        total += np.float64(r["loss"][0, 0])
    return np.float32(total)



# revision 6
# speedup vs baseline: 1.1171x; 1.1171x over previous
"""Chamfer loss kernel for Trainium2, batch-parallel over 8 NeuronCores.

Per core (one batch element b):
  gts = src_points[b] @ R^T + t          (on device, bf16-pair matmul)
  P[i,j] = |gts_i|^2 + |recon_j|^2 - 2 gts_i . recon_j
  loss_b = sum_j min_i P + sum_i min_j P
Host sums the 8 partial losses.

Structure (v3):
- ONE K=11 bf16 matmul produces the COMPLETE distance tile in PSUM:
  rows 0-8 are the -2*g.p cross terms in a bf16 hi/lo pair decomposition
  (hi*hi + hi*lo + lo*hi, ~2^-18 relative accuracy), row 9 carries the
  yy_j norm (ones x yy), row 10 carries the xx_i norm (xx x ones). The
  norms are single bf16; their error is row/column-structured and cancels
  to ~1e-4 in the summed loss.
- PSUM exit is split across engines: most blocks leave PSUM through the
  ACT engine (Identity activation -> bf16 SBUF); a tuned fraction leaves
  through DVE tensor_scalar, which stages to bf16 AND accumulates that
  block's per-row min in the same instruction (accum_out).
- Per-row mins of ACT-staged blocks use tensor_scalar with a min
  accum_out on the bf16 tile: 4x DVE mode, 0.25 cyc/elem.
- The running per-column min is an elementwise bf16 min; the 32 block
  merges are split between DVE (2x tensor_tensor) and GPSIMD (its own
  running tile, merged with DVE's at the end) so the three engines land
  nearly equally loaded.
- Per-column mins are finished with PE transposes + free-axis folds, and
  everything is summed with a final ones-matmul across partitions.
"""

import os

# the axon client here has no NTFF profile hook; a stray BASS_TRACE=1 in the
# environment would crash run_bass_kernel_spmd on a missing import
os.environ["BASS_NEVER_TRACE"] = "1"

import ml_dtypes
import numpy as np

import concourse.bacc as bacc
import concourse.bass as bass
import concourse.mybir as mybir
import concourse.tile as tile
from concourse.bass_utils import run_bass_kernel_spmd

F32 = mybir.dt.float32
BF16 = mybir.dt.bfloat16
ALU = mybir.AluOpType
AX = mybir.AxisListType
AF = mybir.ActivationFunctionType

N_CORES = 8
NPTS = 4096          # points per set (both gts and recon)
NBLK = NPTS // 128   # 32 row blocks
HALF = 2048          # P tile free width (4 PSUM banks)
BIG = 3.0e38         # min-identity seed

# engine split (tuned against the cost-model timeline)
DVE_EXIT = frozenset((10, 21, 31))   # blocks staged by DVE instead of ACT

_CACHE = {}
LAST_RESULTS = None


def _build_kernel():
    nc = bacc.Bacc("TRN2", target_bir_lowering=False, debug=False)

    srcT = nc.declare_dram_parameter("srcT", [4, NPTS], F32, isOutput=False)
    reconT = nc.declare_dram_parameter("reconT", [4, NPTS], F32, isOutput=False)
    taug = nc.declare_dram_parameter("taug", [4, 4], F32, isOutput=False)
    ident = nc.declare_dram_parameter("ident", [128, 128], BF16, isOutput=False)
    cnorm = nc.declare_dram_parameter("cnorm", [8, 2], BF16, isOutput=False)
    cscal = nc.declare_dram_parameter("cscal", [8, 1], F32, isOutput=False)
    cones = nc.declare_dram_parameter("cones", [128, 1], F32, isOutput=False)
    loss = nc.declare_dram_parameter("loss", [1, 1], F32, isOutput=True)

    with tile.TileContext(nc) as tc:
        with tc.tile_pool(name="sb", bufs=1) as sb:
            prep_pool = tc.alloc_tile_pool(name="prep", bufs=1)
            # ---- phase 0: load inputs (chunked over DMA queues) ---------
            pts = prep_pool.tile([8, NPTS], F32) # rows 0-3 gts_aug, 4-7 recon_aug
            for c in range(4):
                cs = slice(c * 1024, (c + 1) * 1024)
                nc.sync.dma_start(out=pts[0:4, cs], in_=srcT[:, cs])
                nc.sync.dma_start(out=pts[4:8, cs], in_=reconT[:, cs])

            taug_sb = sb.tile([4, 4], F32)
            nc.sync.dma_start(out=taug_sb[:, :], in_=taug[:, :])
            ident_sb = sb.tile([128, 128], BF16)
            nc.sync.dma_start(out=ident_sb[:, :], in_=ident[:, :])
            norm_ones = sb.tile([8, 2], BF16)
            nc.sync.dma_start(out=norm_ones[:, :], in_=cnorm[:, :])
            scal = sb.tile([8, 1], F32)
            nc.sync.dma_start(out=scal[:, :], in_=cscal[:, :])
            ones128 = sb.tile([128, 1], F32)
            nc.sync.dma_start(out=ones128[:, :], in_=cones[:, :])

            # running reduction state, seeded while inputs load
            mrunD = sb.tile([128, NPTS], BF16)   # running col-min
            rminF = sb.tile([128, NBLK], F32)    # full-row mins (ACT blocks)
            rminA = sb.tile([128, NBLK], F32)    # half-row mins (DVE blocks)
            rminB = sb.tile([128, NBLK], F32)
            nc.vector.memset(mrunD[:, :], BIG)
            nc.vector.memset(rminF[:, :], BIG)
            nc.vector.memset(rminA[:, :], BIG)
            nc.vector.memset(rminB[:, :], BIG)

            # PE warm-up: tiny matmuls on the identity while inputs load,
            # so the transform/norm matmuls run at full PE clock
            with tc.tile_pool(name="warm_ps", bufs=1, space="PSUM") as wpp:
                warm_ps = wpp.tile([128, 128], F32)
                for _ in range(40):
                    nc.tensor.matmul(warm_ps[:, :], lhsT=ident_sb[:, :],
                                     rhs=ident_sb[:, :], start=True,
                                     stop=True)

            # ---- phase 1: operand prep ----------------------------------
            # bf16 hi/lo of the transform and of the source points
            th = sb.tile([4, 4], BF16)
            tl = sb.tile([4, 4], BF16)
            nc.vector.tensor_copy(th[:, :], taug_sb[:, :])
            nc.vector.scalar_tensor_tensor(tl[:, :], taug_sb[:, :], 1.0,
                                           th[:, :], ALU.mult, ALU.subtract)
            s_hi = prep_pool.tile([4, NPTS], BF16)
            s_lo = prep_pool.tile([4, NPTS], BF16)
            nc.vector.tensor_copy(s_hi[:, :], pts[0:4, :])
            nc.vector.scalar_tensor_tensor(s_lo[:, :], pts[0:4, :], 1.0,
                                           s_hi[:, :], ALU.mult, ALU.subtract)
            tlhs = sb.tile([12, 4], BF16)
            nc.sync.dma_start(out=tlhs[0:4, :], in_=th[:, :])
            nc.sync.dma_start(out=tlhs[4:8, :], in_=th[:, :])
            nc.sync.dma_start(out=tlhs[8:12, :], in_=tl[:, :])
            trhs = prep_pool.tile([12, NPTS], BF16)
            nc.sync.dma_start(out=trhs[0:4, :], in_=s_hi[:, :])
            nc.sync.dma_start(out=trhs[4:8, :], in_=s_lo[:, :])
            nc.sync.dma_start(out=trhs[8:12, :], in_=s_hi[:, :])

            # squares in single bf16: the staged distance tiles are bf16
            # anyway, and norm errors are row/column-structured, so norm
            # accuracy at bf16 level is provably negligible for the loss
            sqb = prep_pool.tile([8, NPTS], BF16)
            nxy = prep_pool.tile([2, NPTS], BF16)
            # full 8 rows (ACT needs 32-aligned partition bases); the src
            # rows squared here are dummies, overwritten from gts below
            nc.scalar.activation(sqb[:, :], pts[:, :], AF.Square)

            # transform: gts^T rows 0-2 (+ intact ones row 3)
            with tc.tile_pool(name="gts_ps", bufs=1, space="PSUM") as gpp:
                gts_ps = gpp.tile([4, NPTS], F32)
                for c in range(NPTS // 512):
                    cs = slice(c * 512, (c + 1) * 512)
                    nc.tensor.matmul(gts_ps[:, cs], lhsT=tlhs[:, :],
                                     rhs=trhs[:, cs], start=True, stop=True)
                nc.scalar.copy(pts[0:4, :], gts_ps[:, :])
                nc.scalar.activation(sqb[0:4, :], gts_ps[:, :], AF.Square)

            # bf16 hi/lo of (-2*gts | recon)
            c_hi = prep_pool.tile([8, NPTS], BF16)
            c_lo = prep_pool.tile([8, NPTS], BF16)
            nc.vector.tensor_scalar(c_hi[:, :], pts[:, :], scal[:, :], None,
                                    ALU.mult)
            nc.vector.scalar_tensor_tensor(c_lo[:, :], pts[:, :], scal[:, :],
                                           c_hi[:, :], ALU.mult, ALU.subtract)

            # xx (row 0) and yy (row 1) via one K=8 bf16 ones-matmul
            with tc.tile_pool(name="nrm_ps", bufs=1, space="PSUM") as npp:
                nrm_ps = npp.tile([2, NPTS], F32)
                for c in range(NPTS // 512):
                    cs = slice(c * 512, (c + 1) * 512)
                    nc.tensor.matmul(nrm_ps[:, cs], lhsT=norm_ones[:, :],
                                     rhs=sqb[:, cs], start=True, stop=True)
                nc.scalar.copy(nxy[:, :], nrm_ps[:, :])

            # assemble the K=11 matmul operands (SBUF->SBUF DMA row moves)
            # k 0-2: -2g_hi | p_hi   k 3-5: -2g_hi | p_lo   k 6-8: -2g_lo | p_hi
            # k 9:   1      | yy     k 10:  xx     | 1
            lhs = sb.tile([16, NPTS], BF16)
            rhs = sb.tile([16, NPTS], BF16)
            nc.sync.dma_start(out=lhs[0:3, :], in_=c_hi[0:3, :])
            nc.sync.dma_start(out=lhs[3:6, :], in_=c_hi[0:3, :])
            nc.sync.dma_start(out=lhs[6:9, :], in_=c_lo[0:3, :])
            nc.sync.dma_start(out=lhs[9:10, :], in_=c_hi[3:4, :])   # bf16 ones
            nc.sync.dma_start(out=lhs[10:11, :], in_=nxy[0:1, :])   # xx
            nc.sync.dma_start(out=rhs[0:3, :], in_=c_hi[4:7, :])
            nc.sync.dma_start(out=rhs[3:6, :], in_=c_lo[4:7, :])
            nc.sync.dma_start(out=rhs[6:9, :], in_=c_hi[4:7, :])
            nc.sync.dma_start(out=rhs[9:10, :], in_=nxy[1:2, :])    # yy
            nc.sync.dma_start(out=rhs[10:11, :], in_=c_hi[7:8, :])  # bf16 ones

            prep_pool.release()

            # ---- phase 3: distance tiles + fused min reductions ---------
            junk = sb.tile([128, NPTS], BF16)   # throwaway TS main output

            with tc.tile_pool(name="stage_sb", bufs=6) as stg, \
                 tc.tile_pool(name="main_ps", bufs=2, space="PSUM") as mps:
                for b in range(NBLK):
                    sg = stg.tile([128, NPTS], BF16, tag="SG")
                    lw = lhs[0:11, b * 128:(b + 1) * 128]
                    for h in range(2):
                        pt = mps.tile([128, HALF], F32, tag="P")
                        for s in range(HALF // 512):
                            j0 = h * HALF + s * 512
                            nc.tensor.matmul(
                                pt[:, s * 512:(s + 1) * 512], lhsT=lw,
                                rhs=rhs[0:11, j0:j0 + 512],
                                start=True, stop=True)
                        hs = slice(h * HALF, (h + 1) * HALF)
                        if b in DVE_EXIT:
                            # stage to bf16 + per-row min in one DVE op
                            rm = rminA if h == 0 else rminB
                            nc.vector.tensor_scalar(
                                sg[:, hs], pt[:, :], 0.0, None, ALU.add,
                                ALU.min, accum_out=rm[:, b:b + 1])
                        else:
                            nc.scalar.activation(sg[:, hs], pt[:, :],
                                                 AF.Identity)
                    if b not in DVE_EXIT:
                        # full-row min at 4x DVE rate; main output discarded
                        nc.vector.tensor_scalar(
                            junk[:, :], sg[:, :], 0.0, None, ALU.add,
                            ALU.min, accum_out=rminF[:, b:b + 1])
                    # running col-min merge
                    nc.vector.tensor_tensor(mrunD[:, :], sg[:, :],
                                            mrunD[:, :], ALU.min)

            # ---- phase 4: finishers -------------------------------------
            rsum = sb.tile([128, 1], F32)
            cmin = sb.tile([128, NPTS // 128], F32)
            csum = sb.tile([128, 1], F32)
            tot = sb.tile([128, 1], F32)
            loss_sb = sb.tile([1, 1], F32)

            # combine row-min variants and sum over blocks
            nc.vector.tensor_tensor(rminA[:, :], rminA[:, :], rminB[:, :],
                                    ALU.min)
            nc.vector.tensor_tensor(rminA[:, :], rminA[:, :], rminF[:, :],
                                    ALU.min)
            nc.vector.tensor_reduce(rsum[:, :], rminA[:, :], axis=AX.X,
                                    op=ALU.add)

            with tc.tile_pool(name="fin_ps", bufs=4, space="PSUM") as fps:
                # 8 transposes per PSUM tile, one batched fold per group
                for g in range(NPTS // 1024):
                    tp = fps.tile([128, 1024], BF16, tag="T")
                    for c in range(8):
                        j0 = (g * 8 + c) * 128
                        nc.tensor.transpose(tp[:, c * 128:(c + 1) * 128],
                                            mrunD[:, j0:j0 + 128],
                                            ident_sb[:, :])
                    nc.vector.tensor_reduce(
                        cmin[:, 8 * g:8 * g + 8],
                        tp.rearrange("p (g w) -> p g w", w=128),
                        axis=AX.X, op=ALU.min)
                nc.vector.tensor_reduce(csum[:, :], cmin[:, :], axis=AX.X,
                                        op=ALU.add)
                nc.vector.tensor_tensor(tot[:, :], rsum[:, :], csum[:, :],
                                        ALU.add)

                loss_ps = fps.tile([1, 1], F32, tag="L", bufs=1)
                nc.tensor.matmul(loss_ps[:, :], lhsT=tot[:, :],
                                 rhs=ones128[:, :], start=True, stop=True)
                nc.scalar.copy(loss_sb[:, :], loss_ps[:, :])

            nc.sync.dma_start(out=loss[:, :], in_=loss_sb[:, :])

    nc.compile()
    return nc


def _prep_core_inputs(recon_b, src_b, transform_b):
    src_aug = np.empty((4, NPTS), np.float32)
    src_aug[0:3] = src_b.T
    src_aug[3] = 1.0
    rec_aug = np.empty((4, NPTS), np.float32)
    rec_aug[0:3] = recon_b.T
    rec_aug[3] = 1.0
    R = transform_b[:3, :3]
    t = transform_b[:3, 3]
    ta = np.zeros((4, 4), np.float32)
    ta[0:3, 0:3] = R.T
    ta[3, 0:3] = t
    ta[3, 3] = 1.0
    cnorm = np.zeros((8, 2), np.float32)
    cnorm[0:3, 0] = 1.0    # xx from gts squares
    cnorm[4:7, 1] = 1.0    # yy from recon squares
    cnorm = cnorm.astype(ml_dtypes.bfloat16)
    cscal = np.zeros((8, 1), np.float32)
    cscal[0:3] = -2.0
    cscal[3] = 1.0      # aug row -> bf16 ones source
    cscal[4:7] = 1.0
    cscal[7] = 1.0
    return {
        "srcT": np.ascontiguousarray(src_aug),
        "reconT": np.ascontiguousarray(rec_aug),
        "taug": ta,
        "ident": np.eye(128).astype(ml_dtypes.bfloat16),
        "cnorm": cnorm,
        "cscal": cscal,
        "cones": np.ones((128, 1), np.float32),
    }


def kernel(recon, src_points, transform):
    global LAST_RESULTS
    recon = np.asarray(recon, np.float32)
    src_points = np.asarray(src_points, np.float32)
    transform = np.asarray(transform, np.float32)
    B = recon.shape[0]
    assert B == N_CORES

    if "nc" not in _CACHE:
        _CACHE["nc"] = _build_kernel()
    nc = _CACHE["nc"]

    in_maps = [
        _prep_core_inputs(recon[b], src_points[b], transform[b])
        for b in range(B)
    ]
    res = run_bass_kernel_spmd(nc, in_maps, list(range(N_CORES)))
    LAST_RESULTS = res
    total = np.float64(0.0)
    for r in res.results:
        total += np.float64(r["loss"][0, 0])
    return np.float32(total)


# revision 9
# speedup vs baseline: 1.3361x; 1.1960x over previous
"""Chamfer loss kernel for Trainium2, batch-parallel over 8 NeuronCores.

Per core (one batch element b):
  gts = src_points[b] @ R^T + t          (host, fp64)
  P[i,j] = |gts_i|^2 + |recon_j|^2 - 2 gts_i . recon_j
  loss_b = sum_j min_i P + sum_i min_j P
Host sums the 8 partial losses.

Structure (v4):
- The host assembles the two K=11 bf16 matmul operands directly:
  rows 0-8 are the -2*g.p cross terms in a bf16 hi/lo pair decomposition
  (hi*hi + hi*lo + lo*hi, ~2^-18 relative accuracy), row 9 carries the
  yy_j norm (ones x yy), row 10 carries the xx_i norm (xx x ones). The
  norms are single bf16; their error is row/column-structured and cancels
  to ~1e-4 in the summed loss. The device sees ready operands, so the
  whole on-device prep phase is two ~90KB DMA loads.
- ONE K=11 bf16 matmul per 512-col chunk produces the COMPLETE distance
  tile in PSUM.
- PSUM exit is split across engines: most blocks leave PSUM through the
  ACT engine (Identity activation -> bf16 SBUF); two blocks leave through
  DVE tensor_scalar, which stages to bf16 AND accumulates that block's
  per-row min in the same instruction (accum_out) - tuned so ACT and DVE
  land equally loaded.
- Per-row mins of ACT-staged blocks use tensor_scalar with a min
  accum_out on the bf16 tile: 4x DVE mode, 0.25 cyc/elem.
- The running per-column min is a 2x bf16 tensor_tensor per block.
- Per-column mins are finished with PE transposes + free-axis folds, and
  everything is summed with a final ones-matmul across partitions.
"""

import os

# the axon client here has no NTFF profile hook; a stray BASS_TRACE=1 in the
# environment would crash run_bass_kernel_spmd on a missing import
os.environ["BASS_NEVER_TRACE"] = "1"

import ml_dtypes
import numpy as np

import concourse.bacc as bacc
import concourse.bass as bass
import concourse.mybir as mybir
import concourse.tile as tile
from concourse.bass_utils import run_bass_kernel_spmd

F32 = mybir.dt.float32
BF16 = mybir.dt.bfloat16
ALU = mybir.AluOpType
AX = mybir.AxisListType
AF = mybir.ActivationFunctionType

N_CORES = 8
NPTS = 4096          # points per set (both gts and recon)
NBLK = NPTS // 128   # 32 row blocks
HALF = 2048          # P tile free width (4 PSUM banks)
BIG = 3.0e38         # min-identity seed

# blocks staged by DVE instead of ACT (tuned for ACT/DVE balance)
DVE_EXIT = frozenset((5, 16))

_CACHE = {}
LAST_RESULTS = None


def _build_kernel():
    nc = bacc.Bacc("TRN2", target_bir_lowering=False, debug=False)

    lhsd = nc.declare_dram_parameter("lhsd", [11, NPTS], BF16, isOutput=False)
    rhsd = nc.declare_dram_parameter("rhsd", [11, NPTS], BF16, isOutput=False)
    ident = nc.declare_dram_parameter("ident", [128, 128], BF16, isOutput=False)
    cones = nc.declare_dram_parameter("cones", [128, 1], F32, isOutput=False)
    loss = nc.declare_dram_parameter("loss", [1, 1], F32, isOutput=True)

    with tile.TileContext(nc) as tc:
        with tc.tile_pool(name="sb", bufs=1) as sb:
            # ---- phase 0: load operands (chunked over DMA queues) -------
            lhs = sb.tile([11, NPTS], BF16)
            rhs = sb.tile([11, NPTS], BF16)
            for c in range(4):
                cs = slice(c * 1024, (c + 1) * 1024)
                nc.sync.dma_start(out=lhs[:, cs], in_=lhsd[:, cs])
                nc.sync.dma_start(out=rhs[:, cs], in_=rhsd[:, cs])
            ident_sb = sb.tile([128, 128], BF16)
            nc.sync.dma_start(out=ident_sb[:, :], in_=ident[:, :])
            ones128 = sb.tile([128, 1], F32)
            nc.sync.dma_start(out=ones128[:, :], in_=cones[:, :])

            # running reduction state, seeded while operands load
            mrun = sb.tile([128, NPTS], BF16)    # running col-min
            rminF = sb.tile([128, NBLK], F32)    # full-row mins (ACT blocks)
            rminA = sb.tile([128, NBLK], F32)    # half-row mins (DVE blocks)
            rminB = sb.tile([128, NBLK], F32)
            nc.vector.memset(mrun[:, :], BIG)
            nc.vector.memset(rminF[:, :], BIG)
            nc.vector.memset(rminA[:, :], BIG)
            nc.vector.memset(rminB[:, :], BIG)

            # PE warm-up on the identity while operands load, so the main
            # matmul stream starts at full PE clock
            with tc.tile_pool(name="warm_ps", bufs=1, space="PSUM") as wpp:
                warm_ps = wpp.tile([128, 128], F32)
                for _ in range(24):
                    nc.tensor.matmul(warm_ps[:, :], lhsT=ident_sb[:, :],
                                     rhs=ident_sb[:, :], start=True,
                                     stop=True)

            # ---- phase 1: distance tiles + fused min reductions ---------
            junk = sb.tile([128, NPTS], BF16)   # throwaway TS main output

            with tc.tile_pool(name="stage_sb", bufs=4) as stg, \
                 tc.tile_pool(name="main_ps", bufs=2, space="PSUM") as mps:
                for b in range(NBLK):
                    sg = stg.tile([128, NPTS], BF16, tag="SG")
                    lw = lhs[:, b * 128:(b + 1) * 128]
                    for h in range(2):
                        pt = mps.tile([128, HALF], F32, tag="P")
                        for s in range(HALF // 512):
                            j0 = h * HALF + s * 512
                            nc.tensor.matmul(
                                pt[:, s * 512:(s + 1) * 512], lhsT=lw,
                                rhs=rhs[:, j0:j0 + 512],
                                start=True, stop=True)
                        hs = slice(h * HALF, (h + 1) * HALF)
                        if b in DVE_EXIT:
                            # stage to bf16 + per-row min in one DVE op
                            rm = rminA if h == 0 else rminB
                            nc.vector.tensor_scalar(
                                sg[:, hs], pt[:, :], 0.0, None, ALU.add,
                                ALU.min, accum_out=rm[:, b:b + 1])
                        else:
                            nc.scalar.activation(sg[:, hs], pt[:, :],
                                                 AF.Identity)
                    if b not in DVE_EXIT:
                        # full-row min at 4x DVE rate; main output discarded
                        nc.vector.tensor_scalar(
                            junk[:, :], sg[:, :], 0.0, None, ALU.add,
                            ALU.min, accum_out=rminF[:, b:b + 1])
                    # running col-min merge
                    nc.vector.tensor_tensor(mrun[:, :], sg[:, :],
                                            mrun[:, :], ALU.min)

            # ---- phase 2: finishers -------------------------------------
            rsum = sb.tile([128, 1], F32)
            cmin = sb.tile([128, NPTS // 128], F32)
            csum = sb.tile([128, 1], F32)
            tot = sb.tile([128, 1], F32)
            loss_sb = sb.tile([1, 1], F32)

            # combine row-min variants and sum over blocks
            nc.vector.tensor_tensor(rminA[:, :], rminA[:, :], rminB[:, :],
                                    ALU.min)
            nc.vector.tensor_tensor(rminA[:, :], rminA[:, :], rminF[:, :],
                                    ALU.min)
            nc.vector.tensor_reduce(rsum[:, :], rminA[:, :], axis=AX.X,
                                    op=ALU.add)

            with tc.tile_pool(name="fin_ps", bufs=4, space="PSUM") as fps:
                # 8 transposes per PSUM tile, one batched fold per group
                for g in range(NPTS // 1024):
                    tp = fps.tile([128, 1024], BF16, tag="T")
                    for c in range(8):
                        j0 = (g * 8 + c) * 128
                        nc.tensor.transpose(tp[:, c * 128:(c + 1) * 128],
                                            mrun[:, j0:j0 + 128],
                                            ident_sb[:, :])
                    nc.vector.tensor_reduce(
                        cmin[:, 8 * g:8 * g + 8],
                        tp.rearrange("p (g w) -> p g w", w=128),
                        axis=AX.X, op=ALU.min)
                nc.vector.tensor_reduce(csum[:, :], cmin[:, :], axis=AX.X,
                                        op=ALU.add)
                nc.vector.tensor_tensor(tot[:, :], rsum[:, :], csum[:, :],
                                        ALU.add)

                loss_ps = fps.tile([1, 1], F32, tag="L", bufs=1)
                nc.tensor.matmul(loss_ps[:, :], lhsT=tot[:, :],
                                 rhs=ones128[:, :], start=True, stop=True)
                nc.scalar.copy(loss_sb[:, :], loss_ps[:, :])

            nc.sync.dma_start(out=loss[:, :], in_=loss_sb[:, :])

    nc.compile()
    return nc


def _bf16(x):
    return x.astype(ml_dtypes.bfloat16)


def _prep_core_inputs(recon_b, src_b, transform_b):
    # transform on host at fp64: gts = src @ R^T + t
    R = transform_b[:3, :3].astype(np.float64)
    t = transform_b[:3, 3].astype(np.float64)
    gts = src_b.astype(np.float64) @ R.T + t            # [N, 3]
    rec = recon_b.astype(np.float64)                    # [M, 3]

    xx = np.sum(gts * gts, axis=1)                      # [N]
    yy = np.sum(rec * rec, axis=1)                      # [M]

    # bf16 hi/lo pair decomposition of the cross-term factors
    g2 = (-2.0 * gts).astype(np.float32)                # [N, 3]
    g_hi = _bf16(g2)
    g_lo = _bf16(g2 - g_hi.astype(np.float32))
    p32 = rec.astype(np.float32)
    p_hi = _bf16(p32)
    p_lo = _bf16(p32 - p_hi.astype(np.float32))

    lhs = np.empty((11, NPTS), ml_dtypes.bfloat16)
    rhs = np.empty((11, NPTS), ml_dtypes.bfloat16)
    lhs[0:3] = g_hi.T
    lhs[3:6] = g_hi.T
    lhs[6:9] = g_lo.T
    lhs[9] = ml_dtypes.bfloat16(1.0)
    lhs[10] = _bf16(xx.astype(np.float32))
    rhs[0:3] = p_hi.T
    rhs[3:6] = p_lo.T
    rhs[6:9] = p_hi.T
    rhs[9] = _bf16(yy.astype(np.float32))
    rhs[10] = ml_dtypes.bfloat16(1.0)

    return {
        "lhsd": np.ascontiguousarray(lhs),
        "rhsd": np.ascontiguousarray(rhs),
        "ident": np.eye(128).astype(ml_dtypes.bfloat16),
        "cones": np.ones((128, 1), np.float32),
    }


def kernel(recon, src_points, transform):
    global LAST_RESULTS
    recon = np.asarray(recon, np.float32)
    src_points = np.asarray(src_points, np.float32)
    transform = np.asarray(transform, np.float32)
    B = recon.shape[0]
    assert B == N_CORES

    if "nc" not in _CACHE:
        _CACHE["nc"] = _build_kernel()
    nc = _CACHE["nc"]

    in_maps = [
        _prep_core_inputs(recon[b], src_points[b], transform[b])
        for b in range(B)
    ]
    res = run_bass_kernel_spmd(nc, in_maps, list(range(N_CORES)))
    LAST_RESULTS = res
    total = np.float64(0.0)
    for r in res.results:
        total += np.float64(r["loss"][0, 0])
    return np.float32(total)


# revision 13
# speedup vs baseline: 1.3466x; 1.0078x over previous
"""Chamfer loss kernel for Trainium2, batch-parallel over 8 NeuronCores.

Per core (one batch element b):
  gts = src_points[b] @ R^T + t          (host, fp64)
  P[i,j] = |gts_i|^2 + |recon_j|^2 - 2 gts_i . recon_j
  loss_b = sum_j min_i P + sum_i min_j P
Host sums the 8 partial losses.

Structure (v4):
- The host assembles the two K=11 bf16 matmul operands directly:
  rows 0-8 are the -2*g.p cross terms in a bf16 hi/lo pair decomposition
  (hi*hi + hi*lo + lo*hi, ~2^-18 relative accuracy), row 9 carries the
  yy_j norm (ones x yy), row 10 carries the xx_i norm (xx x ones). The
  norms are single bf16; their error is row/column-structured and cancels
  to ~1e-4 in the summed loss. The device sees ready operands, so the
  whole on-device prep phase is two ~90KB DMA loads.
- ONE K=11 bf16 matmul per 512-col chunk produces the COMPLETE distance
  tile in PSUM.
- PSUM exit is split across engines: most blocks leave PSUM through the
  ACT engine (Identity activation -> bf16 SBUF); two blocks leave through
  DVE tensor_scalar, which stages to bf16 AND accumulates that block's
  per-row min in the same instruction (accum_out) - tuned so ACT and DVE
  land equally loaded.
- Per-row mins of ACT-staged blocks use tensor_scalar with a min
  accum_out on the bf16 tile: 4x DVE mode, 0.25 cyc/elem.
- The running per-column min is a 2x bf16 tensor_tensor per block.
- Per-column mins are finished with PE transposes + free-axis folds, and
  everything is summed with a final ones-matmul across partitions.
"""

import os

# the axon client here has no NTFF profile hook; a stray BASS_TRACE=1 in the
# environment would crash run_bass_kernel_spmd on a missing import
os.environ["BASS_NEVER_TRACE"] = "1"

import ml_dtypes
import numpy as np

import concourse.bacc as bacc
import concourse.bass as bass
import concourse.mybir as mybir
import concourse.tile as tile
from concourse.bass_utils import run_bass_kernel_spmd

F32 = mybir.dt.float32
BF16 = mybir.dt.bfloat16
ALU = mybir.AluOpType
AX = mybir.AxisListType
AF = mybir.ActivationFunctionType

N_CORES = 8
NPTS = 4096          # points per set (both gts and recon)
NBLK = NPTS // 128   # 32 row blocks
HALF = 2048          # P tile free width (4 PSUM banks)
BIG = 3.0e38         # min-identity seed

# blocks staged by DVE instead of ACT (tuned for ACT/DVE balance)
DVE_EXIT = frozenset((16,))

_CACHE = {}
LAST_RESULTS = None


def _build_kernel():
    nc = bacc.Bacc("TRN2", target_bir_lowering=False, debug=False)

    lhsd = nc.declare_dram_parameter("lhsd", [11, NPTS], BF16, isOutput=False)
    rhsd = nc.declare_dram_parameter("rhsd", [11, NPTS], BF16, isOutput=False)
    ident = nc.declare_dram_parameter("ident", [128, 128], BF16, isOutput=False)
    cones = nc.declare_dram_parameter("cones", [128, 1], F32, isOutput=False)
    loss = nc.declare_dram_parameter("loss", [1, 1], F32, isOutput=True)

    with tile.TileContext(nc) as tc:
        with tc.tile_pool(name="sb", bufs=1) as sb:
            # ---- phase 0: load operands (chunked over DMA queues) -------
            # first chunks are small so the first distance matmuls (which
            # need lhs cols 0:128 + rhs cols 0:2048) can start early
            lhs = sb.tile([11, NPTS], BF16)
            rhs = sb.tile([11, NPTS], BF16)
            nc.sync.dma_start(out=rhs[:, 0:1024], in_=rhsd[:, 0:1024])
            nc.sync.dma_start(out=lhs[:, 0:1024], in_=lhsd[:, 0:1024])
            for c in range(1, 4):
                cs = slice(c * 1024, (c + 1) * 1024)
                nc.sync.dma_start(out=rhs[:, cs], in_=rhsd[:, cs])
                nc.sync.dma_start(out=lhs[:, cs], in_=lhsd[:, cs])
            ident_sb = sb.tile([128, 128], BF16)
            nc.sync.dma_start(out=ident_sb[:, :], in_=ident[:, :])
            ones128 = sb.tile([128, 1], F32)
            nc.sync.dma_start(out=ones128[:, :], in_=cones[:, :])

            # running reduction state, seeded while operands load
            # (mrun needs no seed: block 0 copies into it)
            mrun = sb.tile([128, NPTS], BF16)    # running col-min
            rminF = sb.tile([128, NBLK], F32)    # full-row mins (ACT blocks)
            rminA = sb.tile([128, NBLK], F32)    # half-row mins (DVE blocks)
            rminB = sb.tile([128, NBLK], F32)
            nc.vector.memset(rminF[:, :], BIG)
            nc.vector.memset(rminA[:, :], BIG)
            nc.vector.memset(rminB[:, :], BIG)

            # PE warm-up on the identity while operands load, so the main
            # matmul stream starts at full PE clock
            with tc.tile_pool(name="warm_ps", bufs=1, space="PSUM") as wpp:
                warm_ps = wpp.tile([128, 128], F32)
                for _ in range(24):
                    nc.tensor.matmul(warm_ps[:, :], lhsT=ident_sb[:, :],
                                     rhs=ident_sb[:, :], start=True,
                                     stop=True)

            # ---- phase 1: distance tiles + fused min reductions ---------
            junk = sb.tile([128, NPTS], BF16)   # throwaway TS main output

            with tc.tile_pool(name="stage_sb", bufs=4) as stg, \
                 tc.tile_pool(name="main_ps", bufs=2, space="PSUM") as mps:
                for b in range(NBLK):
                    sg = stg.tile([128, NPTS], BF16, tag="SG")
                    lw = lhs[:, b * 128:(b + 1) * 128]
                    for h in range(2):
                        pt = mps.tile([128, HALF], F32, tag="P")
                        for s in range(HALF // 512):
                            j0 = h * HALF + s * 512
                            nc.tensor.matmul(
                                pt[:, s * 512:(s + 1) * 512], lhsT=lw,
                                rhs=rhs[:, j0:j0 + 512],
                                start=True, stop=True)
                        hs = slice(h * HALF, (h + 1) * HALF)
                        if b in DVE_EXIT:
                            # stage to bf16 + per-row min in one DVE op
                            rm = rminA if h == 0 else rminB
                            nc.vector.tensor_scalar(
                                sg[:, hs], pt[:, :], 0.0, None, ALU.add,
                                ALU.min, accum_out=rm[:, b:b + 1])
                        else:
                            nc.scalar.activation(sg[:, hs], pt[:, :],
                                                 AF.Identity)
                    if b not in DVE_EXIT:
                        # full-row min at 4x DVE rate; main output discarded
                        nc.vector.tensor_scalar(
                            junk[:, :], sg[:, :], 0.0, None, ALU.add,
                            ALU.min, accum_out=rminF[:, b:b + 1])
                    # running col-min merge (block 0 seeds the run)
                    if b == 0:
                        nc.vector.tensor_copy(mrun[:, :], sg[:, :])
                    else:
                        nc.vector.tensor_tensor(mrun[:, :], sg[:, :],
                                                mrun[:, :], ALU.min)

            # ---- phase 2: finishers -------------------------------------
            rsum = sb.tile([128, 1], F32)
            cmin = sb.tile([128, NPTS // 128], F32)
            csum = sb.tile([128, 1], F32)
            tot = sb.tile([128, 1], F32)
            loss_sb = sb.tile([1, 1], F32)

            # combine row-min variants and sum over blocks
            nc.vector.tensor_tensor(rminA[:, :], rminA[:, :], rminB[:, :],
                                    ALU.min)
            nc.vector.tensor_tensor(rminA[:, :], rminA[:, :], rminF[:, :],
                                    ALU.min)
            nc.vector.tensor_reduce(rsum[:, :], rminA[:, :], axis=AX.X,
                                    op=ALU.add)

            with tc.tile_pool(name="fin_ps", bufs=4, space="PSUM") as fps:
                # 8 transposes per PSUM tile, one batched fold per group
                for g in range(NPTS // 1024):
                    tp = fps.tile([128, 1024], BF16, tag="T")
                    for c in range(8):
                        j0 = (g * 8 + c) * 128
                        nc.tensor.transpose(tp[:, c * 128:(c + 1) * 128],
                                            mrun[:, j0:j0 + 128],
                                            ident_sb[:, :])
                    nc.vector.tensor_reduce(
                        cmin[:, 8 * g:8 * g + 8],
                        tp.rearrange("p (g w) -> p g w", w=128),
                        axis=AX.X, op=ALU.min)
                nc.vector.tensor_reduce(csum[:, :], cmin[:, :], axis=AX.X,
                                        op=ALU.add)
                nc.vector.tensor_tensor(tot[:, :], rsum[:, :], csum[:, :],
                                        ALU.add)

                loss_ps = fps.tile([1, 1], F32, tag="L", bufs=1)
                nc.tensor.matmul(loss_ps[:, :], lhsT=tot[:, :],
                                 rhs=ones128[:, :], start=True, stop=True)
                nc.scalar.copy(loss_sb[:, :], loss_ps[:, :])

            nc.sync.dma_start(out=loss[:, :], in_=loss_sb[:, :])

    nc.compile()
    return nc


def _bf16(x):
    return x.astype(ml_dtypes.bfloat16)


def _prep_core_inputs(recon_b, src_b, transform_b):
    # transform on host at fp64: gts = src @ R^T + t
    R = transform_b[:3, :3].astype(np.float64)
    t = transform_b[:3, 3].astype(np.float64)
    gts = src_b.astype(np.float64) @ R.T + t            # [N, 3]
    rec = recon_b.astype(np.float64)                    # [M, 3]

    xx = np.sum(gts * gts, axis=1)                      # [N]
    yy = np.sum(rec * rec, axis=1)                      # [M]

    # bf16 hi/lo pair decomposition of the cross-term factors
    g2 = (-2.0 * gts).astype(np.float32)                # [N, 3]
    g_hi = _bf16(g2)
    g_lo = _bf16(g2 - g_hi.astype(np.float32))
    p32 = rec.astype(np.float32)
    p_hi = _bf16(p32)
    p_lo = _bf16(p32 - p_hi.astype(np.float32))

    lhs = np.empty((11, NPTS), ml_dtypes.bfloat16)
    rhs = np.empty((11, NPTS), ml_dtypes.bfloat16)
    lhs[0:3] = g_hi.T
    lhs[3:6] = g_hi.T
    lhs[6:9] = g_lo.T
    lhs[9] = ml_dtypes.bfloat16(1.0)
    lhs[10] = _bf16(xx.astype(np.float32))
    rhs[0:3] = p_hi.T
    rhs[3:6] = p_lo.T
    rhs[6:9] = p_hi.T
    rhs[9] = _bf16(yy.astype(np.float32))
    rhs[10] = ml_dtypes.bfloat16(1.0)

    return {
        "lhsd": np.ascontiguousarray(lhs),
        "rhsd": np.ascontiguousarray(rhs),
        "ident": np.eye(128).astype(ml_dtypes.bfloat16),
        "cones": np.ones((128, 1), np.float32),
    }


def kernel(recon, src_points, transform):
    global LAST_RESULTS
    recon = np.asarray(recon, np.float32)
    src_points = np.asarray(src_points, np.float32)
    transform = np.asarray(transform, np.float32)
    B = recon.shape[0]
    assert B == N_CORES

    if "nc" not in _CACHE:
        _CACHE["nc"] = _build_kernel()
    nc = _CACHE["nc"]

    in_maps = [
        _prep_core_inputs(recon[b], src_points[b], transform[b])
        for b in range(B)
    ]
    res = run_bass_kernel_spmd(nc, in_maps, list(range(N_CORES)))
    LAST_RESULTS = res
    total = np.float64(0.0)
    for r in res.results:
        total += np.float64(r["loss"][0, 0])
    return np.float32(total)


# revision 16
# speedup vs baseline: 1.4311x; 1.0628x over previous
"""Chamfer loss kernel for Trainium2, batch-parallel over 8 NeuronCores.

Per core (one batch element b):
  gts = src_points[b] @ R^T + t          (host, fp64)
  P[i,j] = |gts_i|^2 + |recon_j|^2 - 2 gts_i . recon_j
  loss_b = sum_j min_i P + sum_i min_j P
Host sums the 8 partial losses.

Structure (v4):
- The host assembles the two K=11 bf16 matmul operands directly:
  rows 0-8 are the -2*g.p cross terms in a bf16 hi/lo pair decomposition
  (hi*hi + hi*lo + lo*hi, ~2^-18 relative accuracy), row 9 carries the
  yy_j norm (ones x yy), row 10 carries the xx_i norm (xx x ones). The
  norms are single bf16; their error is row/column-structured and cancels
  to ~1e-4 in the summed loss. The device sees ready operands, so the
  whole on-device prep phase is two ~90KB DMA loads.
- ONE K=11 bf16 matmul per 512-col chunk produces the COMPLETE distance
  tile in PSUM.
- PSUM exit is split across engines: most blocks leave PSUM through the
  ACT engine (Identity activation -> bf16 SBUF); two blocks leave through
  DVE tensor_scalar, which stages to bf16 AND accumulates that block's
  per-row min in the same instruction (accum_out) - tuned so ACT and DVE
  land equally loaded.
- Per-row mins of ACT-staged blocks use tensor_scalar with a min
  accum_out on the bf16 tile: 4x DVE mode, 0.25 cyc/elem.
- The running per-column min is a 2x bf16 tensor_tensor per block.
- Per-column mins are finished with PE transposes + free-axis folds, and
  everything is summed with a final ones-matmul across partitions.
"""

import os

# the axon client here has no NTFF profile hook; a stray BASS_TRACE=1 in the
# environment would crash run_bass_kernel_spmd on a missing import
os.environ["BASS_NEVER_TRACE"] = "1"

import ml_dtypes
import numpy as np

import concourse.bacc as bacc
import concourse.bass as bass
import concourse.mybir as mybir
import concourse.tile as tile
from concourse.bass_utils import run_bass_kernel_spmd

F32 = mybir.dt.float32
BF16 = mybir.dt.bfloat16
ALU = mybir.AluOpType
AX = mybir.AxisListType
AF = mybir.ActivationFunctionType

N_CORES = 8
NPTS = 4096          # points per set (both gts and recon)
NBLK = NPTS // 128   # 32 row blocks
HALF = 2048          # P tile free width (4 PSUM banks)
BIG = 3.0e38         # min-identity seed

# blocks staged by DVE instead of ACT (tuned for ACT/DVE balance; the
# last block exits via DVE so its row-min fuses into the exit and the
# final dependency chain is shorter)
DVE_EXIT = frozenset((31,))

_CACHE = {}
LAST_RESULTS = None


def _build_kernel():
    nc = bacc.Bacc("TRN2", target_bir_lowering=False, debug=False)

    lhsd = nc.declare_dram_parameter("lhsd", [11, NPTS], BF16, isOutput=False)
    rhsd = nc.declare_dram_parameter("rhsd", [11, NPTS], BF16, isOutput=False)
    ident = nc.declare_dram_parameter("ident", [128, 128], BF16, isOutput=False)
    cones = nc.declare_dram_parameter("cones", [128, 1], F32, isOutput=False)
    loss = nc.declare_dram_parameter("loss", [1, 1], F32, isOutput=True)

    with tile.TileContext(nc) as tc:
        with tc.tile_pool(name="sb", bufs=1) as sb:
            # ---- phase 0: load operands (two parallel DMA queues) -------
            # rhs rides the SP queue, lhs the ACT queue (idle this early);
            # the first distance matmuls need lhs cols 0:128 + rhs 0:2048
            lhs = sb.tile([11, NPTS], BF16)
            rhs = sb.tile([11, NPTS], BF16)
            ident_sb = sb.tile([128, 128], BF16)
            nc.sync.dma_start(out=ident_sb[:, :], in_=ident[:, :])
            nc.sync.dma_start(out=rhs[:, 0:2048], in_=rhsd[:, 0:2048])
            nc.scalar.dma_start(out=lhs[:, 0:2048], in_=lhsd[:, 0:2048])
            nc.sync.dma_start(out=rhs[:, 2048:4096], in_=rhsd[:, 2048:4096])
            nc.scalar.dma_start(out=lhs[:, 2048:4096], in_=lhsd[:, 2048:4096])
            ones128 = sb.tile([128, 1], F32)
            nc.sync.dma_start(out=ones128[:, :], in_=cones[:, :])

            # running reduction state, seeded while operands load
            # (mrun needs no seed: block 0 copies into it)
            mrun = sb.tile([128, NPTS], BF16)    # running col-min
            rminF = sb.tile([128, NBLK], F32)    # full-row mins (ACT blocks)
            rminA = sb.tile([128, NBLK], F32)    # half-row mins (DVE blocks)
            rminB = sb.tile([128, NBLK], F32)
            nc.vector.memset(rminF[:, :], BIG)
            nc.vector.memset(rminA[:, :], BIG)
            nc.vector.memset(rminB[:, :], BIG)

            # PE warm-up on the identity while operands load, so the main
            # matmul stream starts at full PE clock
            with tc.tile_pool(name="warm_ps", bufs=1, space="PSUM") as wpp:
                warm_ps = wpp.tile([128, 128], F32)
                for _ in range(24):
                    nc.tensor.matmul(warm_ps[:, :], lhsT=ident_sb[:, :],
                                     rhs=ident_sb[:, :], start=True,
                                     stop=True)

            # ---- phase 1: distance tiles + fused min reductions ---------
            junk = sb.tile([128, NPTS], BF16)   # throwaway TS main output

            with tc.tile_pool(name="stage_sb", bufs=4) as stg, \
                 tc.tile_pool(name="main_ps", bufs=2, space="PSUM") as mps:
                for b in range(NBLK):
                    sg = stg.tile([128, NPTS], BF16, tag="SG")
                    lw = lhs[:, b * 128:(b + 1) * 128]
                    for h in range(2):
                        pt = mps.tile([128, HALF], F32, tag="P")
                        for s in range(HALF // 512):
                            j0 = h * HALF + s * 512
                            nc.tensor.matmul(
                                pt[:, s * 512:(s + 1) * 512], lhsT=lw,
                                rhs=rhs[:, j0:j0 + 512],
                                start=True, stop=True)
                        hs = slice(h * HALF, (h + 1) * HALF)
                        if b in DVE_EXIT:
                            # stage to bf16 + per-row min in one DVE op
                            rm = rminA if h == 0 else rminB
                            nc.vector.tensor_scalar(
                                sg[:, hs], pt[:, :], 0.0, None, ALU.add,
                                ALU.min, accum_out=rm[:, b:b + 1])
                        else:
                            nc.scalar.activation(sg[:, hs], pt[:, :],
                                                 AF.Identity)
                    if b not in DVE_EXIT:
                        # full-row min at 4x DVE rate; main output discarded
                        nc.vector.tensor_scalar(
                            junk[:, :], sg[:, :], 0.0, None, ALU.add,
                            ALU.min, accum_out=rminF[:, b:b + 1])
                    # running col-min merge (block 0 seeds the run; the
                    # last block merges in column quarters so the finisher
                    # transposes can start on quarter g while quarter g+1
                    # still merges)
                    if b == 0:
                        nc.vector.tensor_copy(mrun[:, :], sg[:, :])
                    elif b == NBLK - 1:
                        for q in range(4):
                            qs = slice(q * 1024, (q + 1) * 1024)
                            nc.vector.tensor_tensor(mrun[:, qs], sg[:, qs],
                                                    mrun[:, qs], ALU.min)
                    else:
                        nc.vector.tensor_tensor(mrun[:, :], sg[:, :],
                                                mrun[:, :], ALU.min)

            # ---- phase 2: finishers -------------------------------------
            rsum = sb.tile([128, 1], F32)
            cmin = sb.tile([128, NPTS // 128], F32)
            csum = sb.tile([128, 1], F32)
            tot = sb.tile([128, 1], F32)
            loss_sb = sb.tile([1, 1], F32)

            # combine row-min variants and sum over blocks
            nc.vector.tensor_tensor(rminA[:, :], rminA[:, :], rminB[:, :],
                                    ALU.min)
            nc.vector.tensor_tensor(rminA[:, :], rminA[:, :], rminF[:, :],
                                    ALU.min)
            nc.vector.tensor_reduce(rsum[:, :], rminA[:, :], axis=AX.X,
                                    op=ALU.add)

            with tc.tile_pool(name="fin_ps", bufs=4, space="PSUM") as fps:
                # 8 transposes per PSUM tile, one batched fold per group
                for g in range(NPTS // 1024):
                    tp = fps.tile([128, 1024], BF16, tag="T")
                    for c in range(8):
                        j0 = (g * 8 + c) * 128
                        nc.tensor.transpose(tp[:, c * 128:(c + 1) * 128],
                                            mrun[:, j0:j0 + 128],
                                            ident_sb[:, :])
                    nc.vector.tensor_reduce(
                        cmin[:, 8 * g:8 * g + 8],
                        tp.rearrange("p (g w) -> p g w", w=128),
                        axis=AX.X, op=ALU.min)
                nc.vector.tensor_reduce(csum[:, :], cmin[:, :], axis=AX.X,
                                        op=ALU.add)
                nc.vector.tensor_tensor(tot[:, :], rsum[:, :], csum[:, :],
                                        ALU.add)

                loss_ps = fps.tile([1, 1], F32, tag="L", bufs=1)
                nc.tensor.matmul(loss_ps[:, :], lhsT=tot[:, :],
                                 rhs=ones128[:, :], start=True, stop=True)
                nc.scalar.copy(loss_sb[:, :], loss_ps[:, :])

            nc.sync.dma_start(out=loss[:, :], in_=loss_sb[:, :])

    nc.compile()
    return nc


def _bf16(x):
    return x.astype(ml_dtypes.bfloat16)


def _prep_core_inputs(recon_b, src_b, transform_b):
    # transform on host at fp64: gts = src @ R^T + t
    R = transform_b[:3, :3].astype(np.float64)
    t = transform_b[:3, 3].astype(np.float64)
    gts = src_b.astype(np.float64) @ R.T + t            # [N, 3]
    rec = recon_b.astype(np.float64)                    # [M, 3]

    xx = np.sum(gts * gts, axis=1)                      # [N]
    yy = np.sum(rec * rec, axis=1)                      # [M]

    # bf16 hi/lo pair decomposition of the cross-term factors
    g2 = (-2.0 * gts).astype(np.float32)                # [N, 3]
    g_hi = _bf16(g2)
    g_lo = _bf16(g2 - g_hi.astype(np.float32))
    p32 = rec.astype(np.float32)
    p_hi = _bf16(p32)
    p_lo = _bf16(p32 - p_hi.astype(np.float32))

    lhs = np.empty((11, NPTS), ml_dtypes.bfloat16)
    rhs = np.empty((11, NPTS), ml_dtypes.bfloat16)
    lhs[0:3] = g_hi.T
    lhs[3:6] = g_hi.T
    lhs[6:9] = g_lo.T
    lhs[9] = ml_dtypes.bfloat16(1.0)
    lhs[10] = _bf16(xx.astype(np.float32))
    rhs[0:3] = p_hi.T
    rhs[3:6] = p_lo.T
    rhs[6:9] = p_hi.T
    rhs[9] = _bf16(yy.astype(np.float32))
    rhs[10] = ml_dtypes.bfloat16(1.0)

    return {
        "lhsd": np.ascontiguousarray(lhs),
        "rhsd": np.ascontiguousarray(rhs),
        "ident": np.eye(128).astype(ml_dtypes.bfloat16),
        "cones": np.ones((128, 1), np.float32),
    }


def kernel(recon, src_points, transform):
    global LAST_RESULTS
    recon = np.asarray(recon, np.float32)
    src_points = np.asarray(src_points, np.float32)
    transform = np.asarray(transform, np.float32)
    B = recon.shape[0]
    assert B == N_CORES

    if "nc" not in _CACHE:
        _CACHE["nc"] = _build_kernel()
    nc = _CACHE["nc"]

    in_maps = [
        _prep_core_inputs(recon[b], src_points[b], transform[b])
        for b in range(B)
    ]
    res = run_bass_kernel_spmd(nc, in_maps, list(range(N_CORES)))
    LAST_RESULTS = res
    total = np.float64(0.0)
    for r in res.results:
        total += np.float64(r["loss"][0, 0])
    return np.float32(total)


# revision 22
# speedup vs baseline: 1.4353x; 1.0029x over previous
"""Chamfer loss kernel for Trainium2, batch-parallel over 8 NeuronCores.

Per core (one batch element b):
  gts = src_points[b] @ R^T + t          (host, fp64)
  P[i,j] = |gts_i|^2 + |recon_j|^2 - 2 gts_i . recon_j
  loss_b = sum_j min_i P + sum_i min_j P
Host sums the 8 partial losses.

Structure (v4):
- The host assembles the two K=11 bf16 matmul operands directly:
  rows 0-8 are the -2*g.p cross terms in a bf16 hi/lo pair decomposition
  (hi*hi + hi*lo + lo*hi, ~2^-18 relative accuracy), row 9 carries the
  yy_j norm (ones x yy), row 10 carries the xx_i norm (xx x ones). The
  norms are single bf16; their error is row/column-structured and cancels
  to ~1e-4 in the summed loss. The device sees ready operands, so the
  whole on-device prep phase is two ~90KB DMA loads.
- ONE K=11 bf16 matmul per 512-col chunk produces the COMPLETE distance
  tile in PSUM.
- PSUM exit is split across engines: most blocks leave PSUM through the
  ACT engine (Identity activation -> bf16 SBUF); two blocks leave through
  DVE tensor_scalar, which stages to bf16 AND accumulates that block's
  per-row min in the same instruction (accum_out) - tuned so ACT and DVE
  land equally loaded.
- Per-row mins of ACT-staged blocks use tensor_scalar with a min
  accum_out on the bf16 tile: 4x DVE mode, 0.25 cyc/elem.
- The running per-column min is a 2x bf16 tensor_tensor per block.
- Per-column mins are finished with PE transposes + free-axis folds, and
  everything is summed with a final ones-matmul across partitions.
"""

import os

# the axon client here has no NTFF profile hook; a stray BASS_TRACE=1 in the
# environment would crash run_bass_kernel_spmd on a missing import
os.environ["BASS_NEVER_TRACE"] = "1"

import ml_dtypes
import numpy as np

import concourse.bacc as bacc
import concourse.bass as bass
import concourse.mybir as mybir
import concourse.tile as tile
from concourse.bass_utils import run_bass_kernel_spmd

F32 = mybir.dt.float32
BF16 = mybir.dt.bfloat16
ALU = mybir.AluOpType
AX = mybir.AxisListType
AF = mybir.ActivationFunctionType

N_CORES = 8
NPTS = 4096          # points per set (both gts and recon)
NBLK = NPTS // 128   # 32 row blocks
HALF = 2048          # P tile free width (4 PSUM banks)
BIG = 3.0e38         # min-identity seed

# blocks staged by DVE instead of ACT (tuned for ACT/DVE balance): block
# 0 fills DVE's pipeline-fill idle, block 31 shortens the tail since its
# row-min fuses into the exit
DVE_EXIT = frozenset((0, 31))

_CACHE = {}
LAST_RESULTS = None


def _build_kernel():
    nc = bacc.Bacc("TRN2", target_bir_lowering=False, debug=False)

    lhsd = nc.declare_dram_parameter("lhsd", [11, NPTS], BF16, isOutput=False)
    rhsd = nc.declare_dram_parameter("rhsd", [11, NPTS], BF16, isOutput=False)
    ident = nc.declare_dram_parameter("ident", [128, 128], BF16, isOutput=False)
    partial = nc.declare_dram_parameter("partial", [128, 2], F32, isOutput=True)

    with tile.TileContext(nc) as tc:
        with tc.tile_pool(name="sb", bufs=1) as sb:
            # ---- phase 0: load operands (two parallel DMA queues) -------
            # rhs rides the SP queue, lhs the ACT queue (idle this early);
            # the first distance matmuls need lhs cols 0:128 + rhs 0:2048
            lhs = sb.tile([11, NPTS], BF16)
            rhs = sb.tile([11, NPTS], BF16)
            ident_sb = sb.tile([128, 128], BF16)
            nc.sync.dma_start(out=ident_sb[:, :], in_=ident[:, :])
            nc.sync.dma_start(out=rhs[:, 0:2048], in_=rhsd[:, 0:2048])
            nc.scalar.dma_start(out=lhs[:, 0:2048], in_=lhsd[:, 0:2048])
            nc.sync.dma_start(out=rhs[:, 2048:4096], in_=rhsd[:, 2048:4096])
            nc.scalar.dma_start(out=lhs[:, 2048:4096], in_=lhsd[:, 2048:4096])

            # running reduction state, seeded while operands load
            # (mrun needs no seed: block 0 copies into it)
            mrun = sb.tile([128, NPTS], BF16)    # running col-min
            rminF = sb.tile([128, NBLK], F32)    # full-row mins (ACT blocks)
            rminA = sb.tile([128, NBLK], F32)    # half-row mins (DVE blocks)
            rminB = sb.tile([128, NBLK], F32)
            nc.vector.memset(rminF[:, :], BIG)
            nc.vector.memset(rminA[:, :], BIG)
            nc.vector.memset(rminB[:, :], BIG)

            # PE warm-up on the identity while operands load, so the main
            # matmul stream starts at full PE clock
            with tc.tile_pool(name="warm_ps", bufs=1, space="PSUM") as wpp:
                warm_ps = wpp.tile([128, 128], F32)
                for _ in range(24):
                    nc.tensor.matmul(warm_ps[:, :], lhsT=ident_sb[:, :],
                                     rhs=ident_sb[:, :], start=True,
                                     stop=True)

            # ---- phase 1: distance tiles + fused min reductions ---------
            junk = sb.tile([128, NPTS], BF16)   # throwaway TS main output

            with tc.tile_pool(name="stage_sb", bufs=4) as stg, \
                 tc.tile_pool(name="main_ps", bufs=2, space="PSUM") as mps:
                for b in range(NBLK):
                    sg = stg.tile([128, NPTS], BF16, tag="SG")
                    lw = lhs[:, b * 128:(b + 1) * 128]
                    for h in range(2):
                        pt = mps.tile([128, HALF], F32, tag="P")
                        for s in range(HALF // 512):
                            j0 = h * HALF + s * 512
                            nc.tensor.matmul(
                                pt[:, s * 512:(s + 1) * 512], lhsT=lw,
                                rhs=rhs[:, j0:j0 + 512],
                                start=True, stop=True)
                        hs = slice(h * HALF, (h + 1) * HALF)
                        if b in DVE_EXIT:
                            # stage to bf16 + per-row min in one DVE op
                            rm = rminA if h == 0 else rminB
                            nc.vector.tensor_scalar(
                                sg[:, hs], pt[:, :], 0.0, None, ALU.add,
                                ALU.min, accum_out=rm[:, b:b + 1])
                        else:
                            nc.scalar.activation(sg[:, hs], pt[:, :],
                                                 AF.Identity)
                    if b not in DVE_EXIT:
                        # full-row min at 4x DVE rate; main output discarded
                        nc.vector.tensor_scalar(
                            junk[:, :], sg[:, :], 0.0, None, ALU.add,
                            ALU.min, accum_out=rminF[:, b:b + 1])
                    # running col-min merge (block 0 seeds the run; the
                    # last block merges in column quarters so the finisher
                    # transposes can start on quarter g while quarter g+1
                    # still merges)
                    if b == 0:
                        nc.vector.tensor_copy(mrun[:, :], sg[:, :])
                    elif b == NBLK - 1:
                        for q in range(4):
                            qs = slice(q * 1024, (q + 1) * 1024)
                            nc.vector.tensor_tensor(mrun[:, qs], sg[:, qs],
                                                    mrun[:, qs], ALU.min)
                    else:
                        nc.vector.tensor_tensor(mrun[:, :], sg[:, :],
                                                mrun[:, :], ALU.min)

            # ---- phase 2: finishers -------------------------------------
            # per-partition sums go to the host, which adds the 256 floats
            psums = sb.tile([128, 2], F32)
            cmin = sb.tile([128, NPTS // 128], F32)

            # combine row-min variants and sum over blocks
            nc.vector.tensor_tensor(rminA[:, :], rminA[:, :], rminB[:, :],
                                    ALU.min)
            nc.vector.tensor_tensor(rminA[:, :], rminA[:, :], rminF[:, :],
                                    ALU.min)
            nc.vector.tensor_reduce(psums[:, 0:1], rminA[:, :], axis=AX.X,
                                    op=ALU.add)

            with tc.tile_pool(name="fin_ps", bufs=4, space="PSUM") as fps:
                # 8 transposes per PSUM tile, one batched fold per group
                for g in range(NPTS // 1024):
                    tp = fps.tile([128, 1024], BF16, tag="T")
                    for c in range(8):
                        j0 = (g * 8 + c) * 128
                        nc.tensor.transpose(tp[:, c * 128:(c + 1) * 128],
                                            mrun[:, j0:j0 + 128],
                                            ident_sb[:, :])
                    nc.vector.tensor_reduce(
                        cmin[:, 8 * g:8 * g + 8],
                        tp.rearrange("p (g w) -> p g w", w=128),
                        axis=AX.X, op=ALU.min)
                nc.vector.tensor_reduce(psums[:, 1:2], cmin[:, :], axis=AX.X,
                                        op=ALU.add)

            nc.sync.dma_start(out=partial[:, :], in_=psums[:, :])

    nc.compile()
    return nc


def _bf16(x):
    return x.astype(ml_dtypes.bfloat16)


def _prep_core_inputs(recon_b, src_b, transform_b):
    # transform on host at fp64: gts = src @ R^T + t
    R = transform_b[:3, :3].astype(np.float64)
    t = transform_b[:3, 3].astype(np.float64)
    gts = src_b.astype(np.float64) @ R.T + t            # [N, 3]
    rec = recon_b.astype(np.float64)                    # [M, 3]

    xx = np.sum(gts * gts, axis=1)                      # [N]
    yy = np.sum(rec * rec, axis=1)                      # [M]

    # bf16 hi/lo pair decomposition of the cross-term factors
    g2 = (-2.0 * gts).astype(np.float32)                # [N, 3]
    g_hi = _bf16(g2)
    g_lo = _bf16(g2 - g_hi.astype(np.float32))
    p32 = rec.astype(np.float32)
    p_hi = _bf16(p32)
    p_lo = _bf16(p32 - p_hi.astype(np.float32))

    lhs = np.empty((11, NPTS), ml_dtypes.bfloat16)
    rhs = np.empty((11, NPTS), ml_dtypes.bfloat16)
    lhs[0:3] = g_hi.T
    lhs[3:6] = g_hi.T
    lhs[6:9] = g_lo.T
    lhs[9] = ml_dtypes.bfloat16(1.0)
    lhs[10] = _bf16(xx.astype(np.float32))
    rhs[0:3] = p_hi.T
    rhs[3:6] = p_lo.T
    rhs[6:9] = p_hi.T
    rhs[9] = _bf16(yy.astype(np.float32))
    rhs[10] = ml_dtypes.bfloat16(1.0)

    return {
        "lhsd": np.ascontiguousarray(lhs),
        "rhsd": np.ascontiguousarray(rhs),
        "ident": np.eye(128).astype(ml_dtypes.bfloat16),
    }


def kernel(recon, src_points, transform):
    global LAST_RESULTS
    recon = np.asarray(recon, np.float32)
    src_points = np.asarray(src_points, np.float32)
    transform = np.asarray(transform, np.float32)
    B = recon.shape[0]
    assert B == N_CORES

    if "nc" not in _CACHE:
        _CACHE["nc"] = _build_kernel()
    nc = _CACHE["nc"]

    in_maps = [
        _prep_core_inputs(recon[b], src_points[b], transform[b])
        for b in range(B)
    ]
    res = run_bass_kernel_spmd(nc, in_maps, list(range(N_CORES)))
    LAST_RESULTS = res
    total = np.float64(0.0)
    for r in res.results:
        total += np.float64(np.sum(r["partial"].astype(np.float64)))
    return np.float32(total)


# revision 26
# speedup vs baseline: 1.4402x; 1.0034x over previous
"""Chamfer loss kernel for Trainium2, batch-parallel over 8 NeuronCores.

Per core (one batch element b):
  gts = src_points[b] @ R^T + t          (host, fp64)
  P[i,j] = |gts_i|^2 + |recon_j|^2 - 2 gts_i . recon_j
  loss_b = sum_j min_i P + sum_i min_j P
Host sums the 8 partial losses.

Structure (v4):
- The host assembles the two K=11 bf16 matmul operands directly:
  rows 0-8 are the -2*g.p cross terms in a bf16 hi/lo pair decomposition
  (hi*hi + hi*lo + lo*hi, ~2^-18 relative accuracy), row 9 carries the
  yy_j norm (ones x yy), row 10 carries the xx_i norm (xx x ones). The
  norms are single bf16; their error is row/column-structured and cancels
  to ~1e-4 in the summed loss. The device sees ready operands, so the
  whole on-device prep phase is two ~90KB DMA loads.
- ONE K=11 bf16 matmul per 512-col chunk produces the COMPLETE distance
  tile in PSUM.
- PSUM exit is split across engines: most blocks leave PSUM through the
  ACT engine (Identity activation -> bf16 SBUF); two blocks leave through
  DVE tensor_scalar, which stages to bf16 AND accumulates that block's
  per-row min in the same instruction (accum_out) - tuned so ACT and DVE
  land equally loaded.
- Per-row mins of ACT-staged blocks use tensor_scalar with a min
  accum_out on the bf16 tile: 4x DVE mode, 0.25 cyc/elem.
- The running per-column min is a 2x bf16 tensor_tensor per block.
- Per-column mins are finished with PE transposes + free-axis folds, and
  everything is summed with a final ones-matmul across partitions.
"""

import os

# the axon client here has no NTFF profile hook; a stray BASS_TRACE=1 in the
# environment would crash run_bass_kernel_spmd on a missing import
os.environ["BASS_NEVER_TRACE"] = "1"

import ml_dtypes
import numpy as np

import concourse.bacc as bacc
import concourse.bass as bass
import concourse.mybir as mybir
import concourse.tile as tile
from concourse.bass_utils import run_bass_kernel_spmd

F32 = mybir.dt.float32
BF16 = mybir.dt.bfloat16
ALU = mybir.AluOpType
AX = mybir.AxisListType
AF = mybir.ActivationFunctionType

N_CORES = 8
NPTS = 4096          # points per set (both gts and recon)
NBLK = NPTS // 128   # 32 row blocks
HALF = 2048          # P tile free width (4 PSUM banks)
BIG = 3.0e38         # min-identity seed

# blocks staged by DVE instead of ACT (tuned for ACT/DVE balance); the
# last block exits via DVE so its row-min fuses into the exit and the
# final dependency chain is shorter
DVE_EXIT = frozenset((31,))

_CACHE = {}
LAST_RESULTS = None


def _build_kernel():
    nc = bacc.Bacc("TRN2", target_bir_lowering=False, debug=False)

    lhsd = nc.declare_dram_parameter("lhsd", [11, NPTS], BF16, isOutput=False)
    rhsd = nc.declare_dram_parameter("rhsd", [11, NPTS], BF16, isOutput=False)
    ident = nc.declare_dram_parameter("ident", [128, 128], BF16, isOutput=False)
    partial = nc.declare_dram_parameter("partial", [128, 2], F32, isOutput=True)

    with tile.TileContext(nc) as tc:
        with tc.tile_pool(name="sb", bufs=1) as sb:
            # ---- phase 0: load operands (two parallel DMA queues) -------
            # rhs rides the SP queue, lhs the ACT queue (idle this early);
            # the first distance matmuls need lhs cols 0:128 + rhs 0:2048
            lhs = sb.tile([11, NPTS], BF16)
            rhs = sb.tile([11, NPTS], BF16)
            ident_sb = sb.tile([128, 128], BF16)
            nc.sync.dma_start(out=ident_sb[:, :], in_=ident[:, :])
            nc.sync.dma_start(out=rhs[:, 0:2048], in_=rhsd[:, 0:2048])
            nc.scalar.dma_start(out=lhs[:, 0:2048], in_=lhsd[:, 0:2048])
            nc.sync.dma_start(out=rhs[:, 2048:4096], in_=rhsd[:, 2048:4096])
            nc.scalar.dma_start(out=lhs[:, 2048:4096], in_=lhsd[:, 2048:4096])

            # running reduction state (mrun needs no seed: block 0 copies
            # into it; rminA/B columns are fully written per block)
            mrun = sb.tile([128, NPTS], BF16)    # running col-min
            rminA = sb.tile([128, NBLK], F32)    # per-block h0 row mins
            rminB = sb.tile([128, NBLK], F32)    # per-block h1 row mins

            # PE warm-up on the identity while operands load, so the main
            # matmul stream starts at full PE clock
            with tc.tile_pool(name="warm_ps", bufs=1, space="PSUM") as wpp:
                warm_ps = wpp.tile([128, 128], F32)
                for _ in range(24):
                    nc.tensor.matmul(warm_ps[:, :], lhsT=ident_sb[:, :],
                                     rhs=ident_sb[:, :], start=True,
                                     stop=True)

            # ---- phase 1: distance tiles + fused min reductions ---------
            junk = sb.tile([128, NPTS], BF16)   # throwaway TS main output

            with tc.tile_pool(name="stage_sb", bufs=4) as stg, \
                 tc.tile_pool(name="main_ps", bufs=2, space="PSUM") as mps:
                for b in range(NBLK):
                    sg = stg.tile([128, NPTS], BF16, tag="SG")
                    lw = lhs[:, b * 128:(b + 1) * 128]
                    for h in range(2):
                        pt = mps.tile([128, HALF], F32, tag="P")
                        for s in range(HALF // 512):
                            j0 = h * HALF + s * 512
                            nc.tensor.matmul(
                                pt[:, s * 512:(s + 1) * 512], lhsT=lw,
                                rhs=rhs[:, j0:j0 + 512],
                                start=True, stop=True)
                        hs = slice(h * HALF, (h + 1) * HALF)
                        rm = rminA if h == 0 else rminB
                        if b in DVE_EXIT:
                            # stage to bf16 + per-row min in one DVE op
                            nc.vector.tensor_scalar(
                                sg[:, hs], pt[:, :], 0.0, None, ALU.add,
                                ALU.min, accum_out=rm[:, b:b + 1])
                        else:
                            nc.scalar.activation(sg[:, hs], pt[:, :],
                                                 AF.Identity)
                            # half-row min at 4x DVE rate, overlapping the
                            # other half's ACT exit; main output discarded
                            nc.vector.tensor_scalar(
                                junk[:, hs], sg[:, hs], 0.0, None, ALU.add,
                                ALU.min, accum_out=rm[:, b:b + 1])
                    # running col-min merge (block 0 seeds the run; the
                    # last block merges in column quarters so the finisher
                    # transposes can start on quarter g while quarter g+1
                    # still merges)
                    if b == 0:
                        nc.vector.tensor_copy(mrun[:, :], sg[:, :])
                    elif b == NBLK - 1:
                        for q in range(4):
                            qs = slice(q * 1024, (q + 1) * 1024)
                            nc.vector.tensor_tensor(mrun[:, qs], sg[:, qs],
                                                    mrun[:, qs], ALU.min)
                    else:
                        nc.vector.tensor_tensor(mrun[:, :], sg[:, :],
                                                mrun[:, :], ALU.min)

            # ---- phase 2: finishers -------------------------------------
            # per-partition sums go to the host, which adds the 256 floats
            psums = sb.tile([128, 2], F32)
            cmin = sb.tile([128, NPTS // 128], F32)

            # combine per-half row mins and sum over blocks
            nc.vector.tensor_tensor(rminA[:, :], rminA[:, :], rminB[:, :],
                                    ALU.min)
            nc.vector.tensor_reduce(psums[:, 0:1], rminA[:, :], axis=AX.X,
                                    op=ALU.add)

            with tc.tile_pool(name="fin_ps", bufs=4, space="PSUM") as fps:
                # 8 transposes per PSUM tile, one batched fold per group
                for g in range(NPTS // 1024):
                    tp = fps.tile([128, 1024], BF16, tag="T")
                    for c in range(8):
                        j0 = (g * 8 + c) * 128
                        nc.tensor.transpose(tp[:, c * 128:(c + 1) * 128],
                                            mrun[:, j0:j0 + 128],
                                            ident_sb[:, :])
                    nc.vector.tensor_reduce(
                        cmin[:, 8 * g:8 * g + 8],
                        tp.rearrange("p (g w) -> p g w", w=128),
                        axis=AX.X, op=ALU.min)
                nc.vector.tensor_reduce(psums[:, 1:2], cmin[:, :], axis=AX.X,
                                        op=ALU.add)

            nc.sync.dma_start(out=partial[:, :], in_=psums[:, :])

    nc.compile()
    return nc


def _bf16(x):
    return x.astype(ml_dtypes.bfloat16)


def _prep_core_inputs(recon_b, src_b, transform_b):
    # transform on host at fp64: gts = src @ R^T + t
    R = transform_b[:3, :3].astype(np.float64)
    t = transform_b[:3, 3].astype(np.float64)
    gts = src_b.astype(np.float64) @ R.T + t            # [N, 3]
    rec = recon_b.astype(np.float64)                    # [M, 3]

    xx = np.sum(gts * gts, axis=1)                      # [N]
    yy = np.sum(rec * rec, axis=1)                      # [M]

    # bf16 hi/lo pair decomposition of the cross-term factors
    g2 = (-2.0 * gts).astype(np.float32)                # [N, 3]
    g_hi = _bf16(g2)
    g_lo = _bf16(g2 - g_hi.astype(np.float32))
    p32 = rec.astype(np.float32)
    p_hi = _bf16(p32)
    p_lo = _bf16(p32 - p_hi.astype(np.float32))

    lhs = np.empty((11, NPTS), ml_dtypes.bfloat16)
    rhs = np.empty((11, NPTS), ml_dtypes.bfloat16)
    lhs[0:3] = g_hi.T
    lhs[3:6] = g_hi.T
    lhs[6:9] = g_lo.T
    lhs[9] = ml_dtypes.bfloat16(1.0)
    lhs[10] = _bf16(xx.astype(np.float32))
    rhs[0:3] = p_hi.T
    rhs[3:6] = p_lo.T
    rhs[6:9] = p_hi.T
    rhs[9] = _bf16(yy.astype(np.float32))
    rhs[10] = ml_dtypes.bfloat16(1.0)

    return {
        "lhsd": np.ascontiguousarray(lhs),
        "rhsd": np.ascontiguousarray(rhs),
        "ident": np.eye(128).astype(ml_dtypes.bfloat16),
    }


def kernel(recon, src_points, transform):
    global LAST_RESULTS
    recon = np.asarray(recon, np.float32)
    src_points = np.asarray(src_points, np.float32)
    transform = np.asarray(transform, np.float32)
    B = recon.shape[0]
    assert B == N_CORES

    if "nc" not in _CACHE:
        _CACHE["nc"] = _build_kernel()
    nc = _CACHE["nc"]

    in_maps = [
        _prep_core_inputs(recon[b], src_points[b], transform[b])
        for b in range(B)
    ]
    res = run_bass_kernel_spmd(nc, in_maps, list(range(N_CORES)))
    LAST_RESULTS = res
    total = np.float64(0.0)
    for r in res.results:
        total += np.float64(np.sum(r["partial"].astype(np.float64)))
    return np.float32(total)
